# revision 1
# baseline (speedup 1.0000x reference)
"""AdaConv2d (per-pixel 3x3 dynamic conv) on 8 TRN2 NeuronCores.

out[b,c,h,w] = sum_t x_pad[b,c,h+dh(t),w+dw(t)] * dk[b,c,t,h,w]

Sharding: pure data parallel over batch (B=8 -> one batch element per core).

Per-core layout: partition p = s*64 + c  (c = channel 0..63, s = H-half 0..1).
Each partition holds its half-plane of x padded to [66 rows x 130 cols] in the
free dim, so all 9 taps become flat free-dim window reads (zero pad absorbs
boundaries). The big dynamic_kernel tensor streams tap by tap.

Engines: DVE computes the 9 per-tap products (f32 in, bf16 out); the tap sum
is accumulated on the TensorEngine via identity-matmul into PSUM (f32); ACT
drains PSUM to SBUF; DMA streams dk/out. Memory-bound: ~46 MB/core HBM traffic.
"""

import numpy as np

from concourse import bacc, bass, tile
from concourse import mybir
from concourse.bass_utils import run_bass_kernel_spmd
from concourse.masks import make_identity

F32 = mybir.dt.float32
BF16 = mybir.dt.bfloat16

B, C, H, W = 8, 64, 128, 128
K = 3
NTAP = K * K
NCORES = 8

HALF = H // 2           # 64 rows per half-plane
NBLK = 2                # row-blocks per half
RB = HALF // NBLK       # 32 rows per block
XROWS = HALF + 2        # 66 padded rows per partition
XCOLS = W + 2           # 130 padded cols

_CACHED_NC = None


def _emit(tc, nc, x_ap, dk_ap, out_ap):
    ctx_pools = []

    def pool(name, bufs, space=bass.MemorySpace.SBUF):
        p = tc.tile_pool(name=name, bufs=bufs, space=space)
        ctx_pools.append(p)
        return p.__enter__()

    try:
        const_pool = pool("const", 1)
        x_pool = pool("xp", 1)
        dk_pool = pool("dk", 4)
        tmp_pool = pool("tmp", 3)
        out_pool = pool("osb", 2)
        psum_pool = pool("ps", 8, space=bass.MemorySpace.PSUM)

        identity = const_pool.tile([128, 128], BF16, name="identity")
        make_identity(nc, identity)

        # Resident padded x: partition (s*64+c) holds x[c] rows of its half
        # with a one-row halo, in a [66, 130] zero-padded frame.
        x_tile = x_pool.tile([128, XROWS, XCOLS], F32, name="x_tile")
        nc.gpsimd.memset(x_tile[:], 0.0)
        # s=0: x rows -1..64 -> frame rows 0..65 (row 0 stays zero)
        nc.sync.dma_start(
            out=x_tile[0:64, 1:XROWS, 1 : W + 1], in_=x_ap[:, 0 : HALF + 1, :]
        )
        # s=1: x rows 63..128 -> frame rows 0..65 (row 65 stays zero)
        nc.sync.dma_start(
            out=x_tile[64:128, 0 : XROWS - 1, 1 : W + 1],
            in_=x_ap[:, HALF - 1 : H, :],
        )

        for b in range(NBLK):
            r0 = b * RB  # first output row of this block within each half
            ps_tiles = [
                psum_pool.tile([128, 4, 128], F32, name=f"ps_{b}_{j}", tag="ps")
                for j in range(RB * W // 512)
            ]
            for t in range(NTAP):
                dh, dw = t // K - 1, t % K - 1
                dk_t = dk_pool.tile([128, RB, W], F32, name="dk_t", tag="dk")
                for s in range(2):
                    h0 = s * HALF + r0
                    nc.sync.dma_start(
                        out=dk_t[s * 64 : s * 64 + 64, :, :],
                        in_=dk_ap[:, t, h0 : h0 + RB, :],
                    )
                tmp = tmp_pool.tile([128, RB, W], BF16, name="tmp", tag="tmp")
                xr = r0 + dh + 1
                nc.vector.tensor_mul(
                    tmp[:],
                    x_tile[:, xr : xr + RB, dw + 1 : dw + 1 + W],
                    dk_t[:],
                )
                for j in range(len(ps_tiles)):
                    nc.tensor.matmul(
                        ps_tiles[j][:],
                        identity[:],
                        tmp[:, 4 * j : 4 * j + 4, :],
                        start=(t == 0),
                        stop=(t == NTAP - 1),
                    )

            out_sb = out_pool.tile([128, RB, W], F32, name="out_sb", tag="osb")
            for j in range(len(ps_tiles)):
                nc.scalar.copy(out=out_sb[:, 4 * j : 4 * j + 4, :], in_=ps_tiles[j][:])
            for s in range(2):
                h0 = s * HALF + r0
                nc.sync.dma_start(
                    out=out_ap[:, h0 : h0 + RB, :],
                    in_=out_sb[s * 64 : s * 64 + 64, :, :],
                )
    finally:
        for p in reversed(ctx_pools):
            p.__exit__(None, None, None)


def build_nc():
    global _CACHED_NC
    if _CACHED_NC is not None:
        return _CACHED_NC
    nc = bacc.Bacc("TRN2", target_bir_lowering=False, debug=False, num_devices=NCORES)
    x_ap = nc.dram_tensor("x", [C, H, W], F32, kind="ExternalInput").ap()
    dk_ap = nc.dram_tensor(
        "dynamic_kernel", [C, NTAP, H, W], F32, kind="ExternalInput"
    ).ap()
    out_ap = nc.dram_tensor("out", [C, H, W], F32, kind="ExternalOutput").ap()
    with tile.TileContext(nc) as tc:
        _emit(tc, nc, x_ap, dk_ap, out_ap)
    nc.compile()
    _CACHED_NC = nc
    return nc


def kernel(x: np.ndarray, dynamic_kernel: np.ndarray) -> np.ndarray:
    nc = build_nc()
    in_maps = [
        {
            "x": np.ascontiguousarray(x[i], dtype=np.float32),
            "dynamic_kernel": np.ascontiguousarray(dynamic_kernel[i], dtype=np.float32),
        }
        for i in range(NCORES)
    ]
    res = run_bass_kernel_spmd(nc, in_maps, core_ids=list(range(NCORES)))
    out = np.stack([res.results[i]["out"] for i in range(NCORES)], axis=0)
    return out.astype(np.float32)


# revision 2
# speedup vs baseline: 1.1655x; 1.1655x over previous
"""AdaConv2d (per-pixel 3x3 dynamic conv) on 8 TRN2 NeuronCores.

out[b,c,h,w] = sum_t x_pad[b,c,h+dh(t),w+dw(t)] * dk[b,c,t,h,w]

Sharding: pure data parallel over batch (B=8 -> one batch element per core).

Per-core layout: partition p = s*64 + c  (c = channel 0..63, s = H-half 0..1).
Each partition holds its half-plane of x padded to [66 rows x 130 cols] in the
free dim, so all 9 taps become flat free-dim window reads (zero pad absorbs
boundaries). The big dynamic_kernel tensor streams tap by tap.

Engines: DVE computes the 9 per-tap products (f32 in, bf16 out); the tap sum
is accumulated on the TensorEngine via identity-matmul into PSUM (f32); ACT
drains PSUM to SBUF; DMA streams dk/out. Memory-bound: ~46 MB/core HBM traffic.
"""

import numpy as np

from concourse import bacc, bass, tile
from concourse import mybir
from concourse.bass_utils import run_bass_kernel_spmd
from concourse.masks import make_identity

F32 = mybir.dt.float32
BF16 = mybir.dt.bfloat16

B, C, H, W = 8, 64, 128, 128
K = 3
NTAP = K * K
NCORES = 8

HALF = H // 2           # 64 rows per half-plane
NBLK = 2                # row-blocks per half
RB = HALF // NBLK       # 32 rows per block
XROWS = HALF + 2        # 66 padded rows per partition
XCOLS = W + 2           # 130 padded cols

_CACHED_NC = None


def _emit(tc, nc, x_ap, dk_ap, out_ap):
    ctx_pools = []

    def pool(name, bufs, space=bass.MemorySpace.SBUF):
        p = tc.tile_pool(name=name, bufs=bufs, space=space)
        ctx_pools.append(p)
        return p.__enter__()

    try:
        const_pool = pool("const", 1)
        x_pool = pool("xp", 1)
        dk_pool = pool("dk", 4)
        tmp_pool = pool("tmp", 3)
        out_pool = pool("osb", 2)
        psum_pool = pool("ps", 8, space=bass.MemorySpace.PSUM)

        identity = const_pool.tile([128, 128], BF16, name="identity")
        make_identity(nc, identity)

        # Resident padded x: partition (s*64+c) holds x[c] rows of its half
        # with a one-row halo, in a [66, 130] zero-padded frame. A strided
        # DMA straight into the frame shatters into 512B descriptors, so
        # stage contiguously and restructure with compute-engine copies.
        x_tile = x_pool.tile([128, XROWS, XCOLS], F32, name="x_tile")
        x_stage = x_pool.tile([128, XROWS - 1, W], F32, name="x_stage")
        nc.gpsimd.memset(x_tile[:], 0.0)
        nc.sync.dma_start(out=x_stage[0:64, :, :], in_=x_ap[:, 0 : HALF + 1, :])
        nc.sync.dma_start(out=x_stage[64:128, :, :], in_=x_ap[:, HALF - 1 : H, :])
        # s=0: x rows -1..64 -> frame rows 0..65 (row 0 stays zero)
        nc.vector.tensor_copy(x_tile[0:64, 1:XROWS, 1 : W + 1], x_stage[0:64, :, :])
        # s=1: x rows 63..128 -> frame rows 0..65 (row 65 stays zero)
        nc.scalar.copy(
            out=x_tile[64:128, 0 : XROWS - 1, 1 : W + 1], in_=x_stage[64:128, :, :]
        )

        for b in range(NBLK):
            r0 = b * RB  # first output row of this block within each half
            ps_tiles = [
                psum_pool.tile([128, 4, 128], F32, name=f"ps_{b}_{j}", tag="ps")
                for j in range(RB * W // 512)
            ]
            for t in range(NTAP):
                dh, dw = t // K - 1, t % K - 1
                dk_t = dk_pool.tile([128, RB, W], F32, name="dk_t", tag="dk")
                for s in range(2):
                    h0 = s * HALF + r0
                    nc.sync.dma_start(
                        out=dk_t[s * 64 : s * 64 + 64, :, :],
                        in_=dk_ap[:, t, h0 : h0 + RB, :],
                    )
                tmp = tmp_pool.tile([128, RB, W], BF16, name="tmp", tag="tmp")
                xr = r0 + dh + 1
                nc.vector.tensor_mul(
                    tmp[:],
                    x_tile[:, xr : xr + RB, dw + 1 : dw + 1 + W],
                    dk_t[:],
                )
                for j in range(len(ps_tiles)):
                    nc.tensor.matmul(
                        ps_tiles[j][:],
                        identity[:],
                        tmp[:, 4 * j : 4 * j + 4, :],
                        start=(t == 0),
                        stop=(t == NTAP - 1),
                    )

            out_sb = out_pool.tile([128, RB, W], F32, name="out_sb", tag="osb")
            for j in range(len(ps_tiles)):
                nc.scalar.copy(out=out_sb[:, 4 * j : 4 * j + 4, :], in_=ps_tiles[j][:])
            for s in range(2):
                h0 = s * HALF + r0
                nc.sync.dma_start(
                    out=out_ap[:, h0 : h0 + RB, :],
                    in_=out_sb[s * 64 : s * 64 + 64, :, :],
                )
    finally:
        for p in reversed(ctx_pools):
            p.__exit__(None, None, None)


def build_nc():
    global _CACHED_NC
    if _CACHED_NC is not None:
        return _CACHED_NC
    nc = bacc.Bacc("TRN2", target_bir_lowering=False, debug=False, num_devices=NCORES)
    x_ap = nc.dram_tensor("x", [C, H, W], F32, kind="ExternalInput").ap()
    dk_ap = nc.dram_tensor(
        "dynamic_kernel", [C, NTAP, H, W], F32, kind="ExternalInput"
    ).ap()
    out_ap = nc.dram_tensor("out", [C, H, W], F32, kind="ExternalOutput").ap()
    with tile.TileContext(nc) as tc:
        _emit(tc, nc, x_ap, dk_ap, out_ap)
    nc.compile()
    _CACHED_NC = nc
    return nc


def kernel(x: np.ndarray, dynamic_kernel: np.ndarray) -> np.ndarray:
    nc = build_nc()
    in_maps = [
        {
            "x": np.ascontiguousarray(x[i], dtype=np.float32),
            "dynamic_kernel": np.ascontiguousarray(dynamic_kernel[i], dtype=np.float32),
        }
        for i in range(NCORES)
    ]
    res = run_bass_kernel_spmd(nc, in_maps, core_ids=list(range(NCORES)))
    out = np.stack([res.results[i]["out"] for i in range(NCORES)], axis=0)
    return out.astype(np.float32)
